# revision 5
# baseline (speedup 1.0000x reference)
"""Trainium2 Bass kernel for nn_MinimalSSM: selective-scan SSM block.

Reference computation (per batch b):
    proj  = x @ W_xproj + b_xproj                # [L, d+2n]
    delta = softplus(proj[:, :d])                # [L, d]
    Bm, Cm = proj[:, d:d+n], proj[:, d+n:]       # [L, n]
    A     = -exp(A_log)                          # [d, n]
    h_t   = exp(delta_t*A) * h_{t-1} + delta_t*Bm_t*x_t   (elementwise [d, n])
    y_t   = sum_n(h_t * Cm_t) + Dp * x_t
    out   = y @ W_out + b_out

Sharding (8 cores): batch (4) x d_model-half (2).  Each core computes the
full recurrence for its 512 channels of its batch, and a partial
out-projection (contraction over its d-half).  A second tiny kernel adds the
two partials per batch (sharded batch x out-column-half).

The time recurrence runs on VectorE's native tensor_tensor_scan
(state = a*state + b along the free dim, fp32 internal state).
exp(delta*A) is computed on ScalarE as activation(Exp, scale=A[:,n]) --
per-partition vector scale -- so the DVE only does the multiplies,
scan, and the n-reduction tree.

Layout inside a core: partition dim = 128-channel block (4 blocks),
free dims = (n=16, t=Tc) per time-chunk, t innermost/contiguous.
"""

import numpy as np
import ml_dtypes

import concourse.bacc as bacc
import concourse.bass as bass
import concourse.tile as tile
from concourse import mybir
from concourse.bass_utils import run_bass_kernel_spmd
from contextlib import ExitStack

F32 = mybir.dt.float32
BF16 = mybir.dt.bfloat16
AF = mybir.ActivationFunctionType
OP = mybir.AluOpType

B, T, D, N = 4, 2048, 1024, 16
DL = D // 2          # channels per core
NJB = DL // 128      # 4 local channel blocks
NKB = D // 128       # 8 contraction blocks for proj
NEB = D // 128       # 8 output-column blocks
PC = 544             # proj columns per core: 512 delta + 16 B + 16 C
TC = 256             # time chunk
NCH = T // TC

_cache = {}


def _build_stage1(t_len=T, tc=TC):
    nch = t_len // tc
    nc = bacc.Bacc("TRN2", target_bir_lowering=False, debug=False, num_devices=8)
    xt = nc.dram_tensor("xt", [D, t_len], BF16, kind="ExternalInput")
    wx = nc.dram_tensor("wx", [D, PC], BF16, kind="ExternalInput")
    bx = nc.dram_tensor("bx", [128, 5], F32, kind="ExternalInput")
    alog = nc.dram_tensor("alog", [128, NJB * N], F32, kind="ExternalInput")
    dp = nc.dram_tensor("dp", [128, NJB], F32, kind="ExternalInput")
    wo = nc.dram_tensor("wo", [DL, D], BF16, kind="ExternalInput")
    bo = nc.dram_tensor("bo", [128, NEB], F32, kind="ExternalInput")
    part = nc.dram_tensor("part", [D, t_len], F32, kind="ExternalOutput")
    bc_dram = nc.dram_tensor("bc_scratch", [t_len // tc, 32, tc], BF16)

    with tile.TileContext(nc) as tc_ctx, ExitStack() as ctx:
        const = ctx.enter_context(tc_ctx.tile_pool(name="const", bufs=1))
        psum = ctx.enter_context(
            tc_ctx.tile_pool(name="psum", bufs=8, space="PSUM"))
        dpool = ctx.enter_context(tc_ctx.tile_pool(name="delta", bufs=6))
        dxpool = ctx.enter_context(tc_ctx.tile_pool(name="dx", bufs=6))
        bcpool = ctx.enter_context(tc_ctx.tile_pool(name="bc", bufs=2))
        reppool = ctx.enter_context(tc_ctx.tile_pool(name="rep", bufs=2))
        apool = ctx.enter_context(tc_ctx.tile_pool(name="apool", bufs=2))
        workpool = ctx.enter_context(tc_ctx.tile_pool(name="work", bufs=3))
        hpool = ctx.enter_context(tc_ctx.tile_pool(name="hpool", bufs=2))
        cpool = ctx.enter_context(tc_ctx.tile_pool(name="carry", bufs=8))
        ypool = ctx.enter_context(tc_ctx.tile_pool(name="y", bufs=6))
        ybfpool = ctx.enter_context(tc_ctx.tile_pool(name="ybf", bufs=8))
        popool = ctx.enter_context(tc_ctx.tile_pool(name="po", bufs=3))

        xt_sb = []
        for kb in range(NKB):
            tt = const.tile([128, t_len], BF16, tag=f"xt{kb}")
            nc.sync.dma_start(tt[:], xt[kb * 128:(kb + 1) * 128, :])
            xt_sb.append(tt)
        wx_sb = []
        for kb in range(NKB):
            tt = const.tile([128, PC], BF16, tag=f"wx{kb}")
            nc.sync.dma_start(tt[:], wx[kb * 128:(kb + 1) * 128, :])
            wx_sb.append(tt)
        wo_sb = []
        for kb in range(NJB):
            tt = const.tile([128, D], BF16, tag=f"wo{kb}")
            nc.sync.dma_start(tt[:], wo[kb * 128:(kb + 1) * 128, :])
            wo_sb.append(tt)
        bx_sb = const.tile([128, 5], F32, tag="bx")
        nc.sync.dma_start(bx_sb[:], bx[:])
        bo_sb = const.tile([128, NEB], F32, tag="bo")
        nc.sync.dma_start(bo_sb[:], bo[:])
        dp_sb = const.tile([128, NJB], F32, tag="dp")
        nc.sync.dma_start(dp_sb[:], dp[:])
        alog_sb = const.tile([128, NJB * N], F32, tag="alog")
        nc.sync.dma_start(alog_sb[:], alog[:])
        aexp_sb = const.tile([128, NJB * N], F32, tag="aexp")
        nc.scalar.activation(aexp_sb[:], alog_sb[:], AF.Exp)
        aneg_sb = const.tile([128, NJB * N], F32, tag="aneg")
        nc.vector.tensor_scalar_mul(aneg_sb[:], aexp_sb[:], -1.0)

        carry_tiles = [None] * NJB
        for ci in range(nch):
            t0 = ci * tc
            delta_tiles = []
            bct = None
            for m in range(5):
                mm = 128 if m < 4 else 32
                ps = psum.tile([mm, tc], F32, tag="ps")
                for kb in range(NKB):
                    nc.tensor.matmul(
                        ps[:],
                        wx_sb[kb][:, m * 128:m * 128 + mm],
                        xt_sb[kb][:, t0:t0 + tc],
                        start=(kb == 0), stop=(kb == NKB - 1))
                if m < 4:
                    # softplus(v) = ln(1 + exp(v)); Exp+Ln share one ACT table
                    et_ = dpool.tile([128, tc], F32, tag="etmp")
                    nc.scalar.activation(et_[:], ps[:], AF.Exp,
                                         bias=bx_sb[:, m:m + 1])
                    dt_ = dpool.tile([128, tc], F32, tag="delta")
                    nc.scalar.activation(dt_[:], et_[:], AF.Ln, bias=1.0)
                    delta_tiles.append(dt_)
                else:
                    bct = bcpool.tile([32, tc], BF16, tag="bc")
                    nc.scalar.activation(bct[:], ps[:], AF.Identity,
                                         bias=bx_sb[:32, 4:5])
            nc.sync.dma_start(bc_dram[ci], bct[:])
            brep = reppool.tile([128, N, tc], BF16, tag="brep")
            crep = reppool.tile([128, N, tc], BF16, tag="crep")
            nc.sync.dma_start(brep[:],
                              bc_dram[ci, 0:N, :].partition_broadcast(128))
            nc.sync.dma_start(crep[:],
                              bc_dram[ci, N:2 * N, :].partition_broadcast(128))

            y_tiles = []
            for jb in range(NJB):
                dt_ = delta_tiles[jb]
                dxt = dxpool.tile([128, tc], BF16, tag="dx")
                nc.vector.tensor_mul(dxt[:], dt_[:], xt_sb[jb][:, t0:t0 + tc])
                at = apool.tile([128, N, tc], F32, tag="a")
                for n in range(N):
                    nc.scalar.activation(
                        at[:, n, :], dt_[:], AF.Exp,
                        scale=aneg_sb[:, jb * N + n:jb * N + n + 1])
                bt = workpool.tile([128, N, tc], BF16, tag="work")
                dx_b = dxt[:].unsqueeze(1).broadcast_to([128, N, tc])
                nc.vector.tensor_mul(bt[:], dx_b, brep[:])
                ht = hpool.tile([128, N, tc], BF16, tag="h")
                for n in range(N):
                    init = 0.0 if ci == 0 else carry_tiles[jb][:, n, :]
                    nc.vector.tensor_tensor_scan(
                        ht[:, n, :], at[:, n, :], bt[:, n, :], init,
                        op0=OP.mult, op1=OP.add)
                newc = cpool.tile([128, N, 1], F32, tag="carry")
                nc.vector.tensor_copy(newc[:], ht[:, :, tc - 1:tc])
                carry_tiles[jb] = newc
                hct = workpool.tile([128, N, tc], BF16, tag="work")
                nc.vector.tensor_mul(hct[:], ht[:], crep[:])
                # n-reduction tree (ping-pong through bt, which is dead)
                nc.vector.tensor_add(bt[:, 0:8, :], hct[:, 0:8, :],
                                     hct[:, 8:16, :])
                nc.vector.tensor_add(bt[:, 8:12, :], bt[:, 0:4, :],
                                     bt[:, 4:8, :])
                nc.vector.tensor_add(bt[:, 12:14, :], bt[:, 8:10, :],
                                     bt[:, 10:12, :])
                yt = ypool.tile([128, tc], F32, tag="y")
                nc.vector.tensor_add(yt[:], bt[:, 12, :], bt[:, 13, :])
                ybt = ybfpool.tile([128, tc], BF16, tag="ybf")
                nc.vector.scalar_tensor_tensor(
                    ybt[:], xt_sb[jb][:, t0:t0 + tc], dp_sb[:, jb:jb + 1],
                    yt[:], op0=OP.mult, op1=OP.add)
                y_tiles.append(ybt)

            for eb in range(NEB):
                pso = psum.tile([128, tc], F32, tag="ps")
                for kb in range(NJB):
                    nc.tensor.matmul(
                        pso[:],
                        wo_sb[kb][:, eb * 128:(eb + 1) * 128],
                        y_tiles[kb][:],
                        start=(kb == 0), stop=(kb == NJB - 1))
                pot = popool.tile([128, tc], F32, tag="po")
                nc.scalar.activation(pot[:], pso[:], AF.Identity,
                                     bias=bo_sb[:, eb:eb + 1])
                nc.sync.dma_start(part[eb * 128:(eb + 1) * 128, t0:t0 + tc],
                                  pot[:])
    nc.compile()
    return nc


def _build_stage2(t_len=T):
    nc = bacc.Bacc("TRN2", target_bir_lowering=False, debug=False, num_devices=8)
    p0 = nc.dram_tensor("p0", [DL, t_len], F32, kind="ExternalInput")
    p1 = nc.dram_tensor("p1", [DL, t_len], F32, kind="ExternalInput")
    s = nc.dram_tensor("s", [DL, t_len], F32, kind="ExternalOutput")
    tcw = 512
    with tile.TileContext(nc) as tc_ctx, ExitStack() as ctx:
        pool = ctx.enter_context(tc_ctx.tile_pool(name="p", bufs=6))
        for kb in range(DL // 128):
            for i in range(t_len // tcw):
                t0 = i * tcw
                a_t = pool.tile([128, tcw], F32, tag="a")
                nc.sync.dma_start(a_t[:], p0[kb * 128:(kb + 1) * 128, t0:t0 + tcw])
                b_t = pool.tile([128, tcw], F32, tag="b")
                nc.sync.dma_start(b_t[:], p1[kb * 128:(kb + 1) * 128, t0:t0 + tcw])
                o_t = pool.tile([128, tcw], F32, tag="o")
                nc.vector.tensor_add(o_t[:], a_t[:], b_t[:])
                nc.sync.dma_start(s[kb * 128:(kb + 1) * 128, t0:t0 + tcw], o_t[:])
    nc.compile()
    return nc


def _stage1_inputs(x, A_log, Dp, W_xproj, b_xproj, W_out, b_out):
    bf = ml_dtypes.bfloat16
    in_maps = []
    for c in range(8):
        b, j = c % 4, c // 4
        lo, hi = j * DL, (j + 1) * DL
        order = np.concatenate(
            [np.arange(lo, hi), np.arange(0, lo), np.arange(hi, D)])
        cols = np.concatenate([np.arange(lo, hi), np.arange(D, D + 2 * N)])
        xt_full = np.ascontiguousarray(x[b].T[order]).astype(bf)
        wxc = np.ascontiguousarray(W_xproj[order][:, cols]).astype(bf)
        bx_pad = np.zeros(5 * 128, np.float32)
        bx_pad[:PC] = b_xproj[cols]
        bx_arr = np.ascontiguousarray(bx_pad.reshape(5, 128).T)
        alog_l = np.ascontiguousarray(
            A_log[lo:hi].reshape(NJB, 128, N).transpose(1, 0, 2).reshape(128, NJB * N))
        dp_l = np.ascontiguousarray(Dp[lo:hi].reshape(NJB, 128).T)
        wo_l = np.ascontiguousarray(W_out[lo:hi]).astype(bf)
        bo_src = b_out if j == 0 else np.zeros_like(b_out)
        bo_l = np.ascontiguousarray(bo_src.reshape(NEB, 128).T.astype(np.float32))
        in_maps.append({
            "xt": xt_full, "wx": wxc, "bx": bx_arr, "alog": alog_l,
            "dp": dp_l, "wo": wo_l, "bo": bo_l,
        })
    return in_maps


def kernel(x, A_log, Dp, W_xproj, b_xproj, W_out, b_out, _trace=False):
    x = np.asarray(x, np.float32)
    A_log = np.asarray(A_log, np.float32)
    Dp = np.asarray(Dp, np.float32)
    W_xproj = np.asarray(W_xproj, np.float32)
    b_xproj = np.asarray(b_xproj, np.float32)
    W_out = np.asarray(W_out, np.float32)
    b_out = np.asarray(b_out, np.float32)

    if "s1" not in _cache:
        _cache["s1"] = _build_stage1()
    if "s2" not in _cache:
        _cache["s2"] = _build_stage2()

    in1 = _stage1_inputs(x, A_log, Dp, W_xproj, b_xproj, W_out, b_out)
    kw = dict(trace=True, trace_cores=list(range(8))) if _trace else {}
    res1 = run_bass_kernel_spmd(_cache["s1"], in1, core_ids=list(range(8)), **kw)
    parts = [res1.results[c]["part"] for c in range(8)]

    in2 = []
    for c in range(8):
        b, eh = c % 4, c // 4
        in2.append({
            "p0": np.ascontiguousarray(parts[b][eh * DL:(eh + 1) * DL]),
            "p1": np.ascontiguousarray(parts[4 + b][eh * DL:(eh + 1) * DL]),
        })
    res2 = run_bass_kernel_spmd(_cache["s2"], in2, core_ids=list(range(8)), **kw)

    outs = []
    for b in range(4):
        s0 = res2.results[b]["s"]
        s1 = res2.results[4 + b]["s"]
        outs.append(np.concatenate([s0, s1], axis=0).T)
    out = np.stack(outs).astype(np.float32)
    if _trace:
        return out, (res1, res2)
    return out


# revision 12
# speedup vs baseline: 1.1401x; 1.1401x over previous
"""Trainium2 Bass kernel for nn_MinimalSSM: selective-scan SSM block.

Reference computation (per batch b):
    proj  = x @ W_xproj + b_xproj                # [L, d+2n]
    delta = softplus(proj[:, :d])                # [L, d]
    Bm, Cm = proj[:, d:d+n], proj[:, d+n:]       # [L, n]
    A     = -exp(A_log)                          # [d, n]
    h_t   = exp(delta_t*A) * h_{t-1} + delta_t*Bm_t*x_t   (elementwise [d, n])
    y_t   = sum_n(h_t * Cm_t) + Dp * x_t
    out   = y @ W_out + b_out

Sharding (8 cores): batch (4) x d_model-half (2).  Each core computes the
full recurrence for its 512 channels of its batch, and a partial
out-projection (contraction over its d-half).  A second tiny kernel adds the
two partials per batch (sharded batch x out-column-half).

The time recurrence runs on VectorE's native tensor_tensor_scan
(state = a*state + b along the free dim, fp32 internal state).
exp(delta*A) is computed on ScalarE as activation(Exp, scale=A[:,n]) --
per-partition vector scale -- so the DVE only does the multiplies,
scan, and the n-reduction tree.

Layout inside a core: partition dim = 128-channel block (4 blocks),
free dims = (n=16, t=Tc) per time-chunk, t innermost/contiguous.
"""

import numpy as np
import ml_dtypes

import concourse.bacc as bacc
import concourse.bass as bass
import concourse.tile as tile
from concourse import mybir
from concourse.bass_utils import run_bass_kernel_spmd
from contextlib import ExitStack

F32 = mybir.dt.float32
BF16 = mybir.dt.bfloat16
AF = mybir.ActivationFunctionType
OP = mybir.AluOpType

B, T, D, N = 4, 2048, 1024, 16
DL = D // 2          # channels per core
NJB = DL // 128      # 4 local channel blocks
NKB = D // 128       # 8 contraction blocks for proj
NEB = D // 128       # 8 output-column blocks
PC = 544             # proj columns per core: 512 delta + 16 B + 16 C
TC = 256             # time chunk
NCH = T // TC

_cache = {}


def _pin_act_tables():
    """Restrict bacc's activation-table choices to the one set containing
    every function we use (Exp, Ln, Identity, MemsetZero) so the compiler
    never inserts mid-kernel ACT_TABLE_LOAD switches."""
    import concourse.bacc as _bacc_mod
    from concourse.hw_specs import get_activation_tables as _orig

    def _only_nl_exp(arch):
        tabs = _orig(arch)
        # keep every entry (act_func_set_id is positional) but empty out the
        # alternatives so the chooser can only pick the one full set
        return {k: (v if k == "natural_log_exp_and_others" else set())
                for k, v in tabs.items()}

    _bacc_mod.get_activation_tables = _only_nl_exp


_pin_act_tables()


def _build_stage1(t_len=T, tc=TC):
    nch = t_len // tc
    nc = bacc.Bacc("TRN2", target_bir_lowering=False, debug=False, num_devices=8)
    xt = nc.dram_tensor("xt", [D, t_len], BF16, kind="ExternalInput")
    wx = nc.dram_tensor("wx", [D, PC], BF16, kind="ExternalInput")
    bx = nc.dram_tensor("bx", [128, 5], F32, kind="ExternalInput")
    alog = nc.dram_tensor("alog", [128, NJB * N], F32, kind="ExternalInput")
    dp = nc.dram_tensor("dp", [128, NJB], F32, kind="ExternalInput")
    wo = nc.dram_tensor("wo", [DL, D], BF16, kind="ExternalInput")
    bo = nc.dram_tensor("bo", [128, NEB], F32, kind="ExternalInput")
    part = nc.dram_tensor("part", [D, t_len], F32, kind="ExternalOutput")
    bc_dram = nc.dram_tensor("bc_scratch", [t_len // tc, 32, tc], BF16)

    with tile.TileContext(nc) as tc_ctx, ExitStack() as ctx:
        const = ctx.enter_context(tc_ctx.tile_pool(name="const", bufs=1))
        psum = ctx.enter_context(
            tc_ctx.tile_pool(name="psum", bufs=8, space="PSUM"))
        dpool = ctx.enter_context(tc_ctx.tile_pool(name="delta", bufs=6))
        dxpool = ctx.enter_context(tc_ctx.tile_pool(name="dx", bufs=6))
        bcpool = ctx.enter_context(tc_ctx.tile_pool(name="bc", bufs=2))
        reppool = ctx.enter_context(tc_ctx.tile_pool(name="rep", bufs=2))
        apool = ctx.enter_context(tc_ctx.tile_pool(name="apool", bufs=2))
        workpool = ctx.enter_context(tc_ctx.tile_pool(name="work", bufs=3))
        hpool = ctx.enter_context(tc_ctx.tile_pool(name="hpool", bufs=2))
        cpool = ctx.enter_context(tc_ctx.tile_pool(name="carry", bufs=8))
        ypool = ctx.enter_context(tc_ctx.tile_pool(name="y", bufs=6))
        ybfpool = ctx.enter_context(tc_ctx.tile_pool(name="ybf", bufs=8))
        popool = ctx.enter_context(tc_ctx.tile_pool(name="po", bufs=3))

        xt_sb = []
        for kb in range(NKB):
            tt = const.tile([128, t_len], BF16, tag=f"xt{kb}")
            nc.sync.dma_start(tt[:], xt[kb * 128:(kb + 1) * 128, :])
            xt_sb.append(tt)
        wx_sb = []
        for kb in range(NKB):
            tt = const.tile([128, PC], BF16, tag=f"wx{kb}")
            nc.sync.dma_start(tt[:], wx[kb * 128:(kb + 1) * 128, :])
            wx_sb.append(tt)
        wo_sb = []
        for kb in range(NJB):
            tt = const.tile([128, D], BF16, tag=f"wo{kb}")
            nc.sync.dma_start(tt[:], wo[kb * 128:(kb + 1) * 128, :])
            wo_sb.append(tt)
        bx_sb = const.tile([128, 5], F32, tag="bx")
        nc.sync.dma_start(bx_sb[:], bx[:])
        bo_sb = const.tile([128, NEB], F32, tag="bo")
        nc.sync.dma_start(bo_sb[:], bo[:])
        dp_sb = const.tile([128, NJB], F32, tag="dp")
        nc.sync.dma_start(dp_sb[:], dp[:])
        alog_sb = const.tile([128, NJB * N], F32, tag="alog")
        nc.sync.dma_start(alog_sb[:], alog[:])
        aexp_sb = const.tile([128, NJB * N], F32, tag="aexp")
        nc.scalar.activation(aexp_sb[:], alog_sb[:], AF.Exp)
        aneg_sb = const.tile([128, NJB * N], F32, tag="aneg")
        nc.vector.tensor_scalar_mul(aneg_sb[:], aexp_sb[:], -1.0)
        # (0, 1) bf16 pattern for the chained-scan dummy columns
        const01 = const.tile([128, N, 2], BF16, tag="const01")
        nc.vector.memset(const01[:, :, 0:1], 0.0)
        nc.vector.memset(const01[:, :, 1:2], 1.0)

        carry_tiles = [None] * NJB
        for ci in range(nch):
            t0 = ci * tc
            delta_tiles = []
            bct = None
            for m in range(5):
                mm = 128 if m < 4 else 32
                ps = psum.tile([mm, tc], F32, tag="ps")
                for kb in range(NKB):
                    nc.tensor.matmul(
                        ps[:],
                        wx_sb[kb][:, m * 128:m * 128 + mm],
                        xt_sb[kb][:, t0:t0 + tc],
                        start=(kb == 0), stop=(kb == NKB - 1))
                if m < 4:
                    # softplus(v) = ln(1 + exp(v)); Exp+Ln share one ACT table
                    et_ = dpool.tile([128, tc], F32, tag="etmp")
                    nc.scalar.activation(et_[:], ps[:], AF.Exp,
                                         bias=bx_sb[:, m:m + 1])
                    dt_ = dpool.tile([128, tc], F32, tag="delta")
                    nc.scalar.activation(dt_[:], et_[:], AF.Ln, bias=1.0)
                    delta_tiles.append(dt_)
                else:
                    bct = bcpool.tile([32, tc], BF16, tag="bc")
                    nc.scalar.activation(bct[:], ps[:], AF.Identity,
                                         bias=bx_sb[:32, 4:5])
            nc.sync.dma_start(bc_dram[ci], bct[:])
            brep = reppool.tile([128, N, tc], BF16, tag="brep")
            crep = reppool.tile([128, N, tc], BF16, tag="crep")
            nc.sync.dma_start(brep[:],
                              bc_dram[ci, 0:N, :].partition_broadcast(128))
            nc.sync.dma_start(crep[:],
                              bc_dram[ci, N:2 * N, :].partition_broadcast(128))

            y_tiles = []
            for jb in range(NJB):
                dt_ = delta_tiles[jb]
                dxt = dxpool.tile([128, tc], BF16, tag="dx")
                nc.vector.tensor_mul(dxt[:], dt_[:], xt_sb[jb][:, t0:t0 + tc])
                # a/b tiles carry 2 leading dummy columns per n-segment:
                # a = (0, 1), b = (carry, 0).  One flat scan then chains all
                # 16 n-segments: the a=0 column resets the running state to
                # the injected carry, the (1, 0) column passes it through.
                tcp = tc + 2
                at = apool.tile([128, N, tcp], BF16, tag="a")
                nc.scalar.activation(at[:, :, 0:2], const01[:], AF.Identity)
                for n in range(N):
                    nc.scalar.activation(
                        at[:, n, 2:], dt_[:], AF.Exp,
                        scale=aneg_sb[:, jb * N + n:jb * N + n + 1])
                bt = workpool.tile([128, N, tcp], BF16, tag="work")
                if ci == 0:
                    nc.scalar.activation(bt[:, :, 0:2], const01[:],
                                         AF.Copy, bias=0.0, scale=0.0)
                else:
                    nc.scalar.activation(bt[:, :, 0:2], carry_tiles[jb][:],
                                         AF.Identity)
                dx_b = dxt[:].unsqueeze(1).broadcast_to([128, N, tc])
                nc.vector.tensor_mul(bt[:, :, 2:], dx_b, brep[:])
                ht = hpool.tile([128, N, tcp], BF16, tag="h")
                nc.vector.tensor_tensor_scan(
                    ht[:].rearrange("p n t -> p (n t)"),
                    at[:].rearrange("p n t -> p (n t)"),
                    bt[:].rearrange("p n t -> p (n t)"),
                    0.0, op0=OP.mult, op1=OP.add)
                if ci < nch - 1:
                    newc = cpool.tile([128, N, 2], BF16, tag="carry")
                    nc.scalar.activation(newc[:, :, 0:1],
                                         ht[:, :, tcp - 1:tcp], AF.Identity)
                    nc.scalar.activation(newc[:, :, 1:2],
                                         ht[:, :, tcp - 1:tcp],
                                         AF.Copy, bias=0.0, scale=0.0)
                    carry_tiles[jb] = newc
                hct = workpool.tile([128, N, tc], BF16, tag="work")
                nc.vector.tensor_mul(hct[:], ht[:, :, 2:], crep[:])
                # n-reduction tree (ping-pong through bt, which is dead)
                nc.vector.tensor_add(bt[:, 0:8, 2:], hct[:, 0:8, :],
                                     hct[:, 8:16, :])
                nc.vector.tensor_add(bt[:, 8:12, 2:], bt[:, 0:4, 2:],
                                     bt[:, 4:8, 2:])
                nc.vector.tensor_add(bt[:, 12:14, 2:], bt[:, 8:10, 2:],
                                     bt[:, 10:12, 2:])
                yt = ypool.tile([128, tc], F32, tag="y")
                nc.vector.tensor_add(yt[:], bt[:, 12, 2:], bt[:, 13, 2:])
                ybt = ybfpool.tile([128, tc], BF16, tag="ybf")
                nc.vector.scalar_tensor_tensor(
                    ybt[:], xt_sb[jb][:, t0:t0 + tc], dp_sb[:, jb:jb + 1],
                    yt[:], op0=OP.mult, op1=OP.add)
                y_tiles.append(ybt)

            for eb in range(NEB):
                pso = psum.tile([128, tc], F32, tag="ps")
                for kb in range(NJB):
                    nc.tensor.matmul(
                        pso[:],
                        wo_sb[kb][:, eb * 128:(eb + 1) * 128],
                        y_tiles[kb][:],
                        start=(kb == 0), stop=(kb == NJB - 1))
                pot = popool.tile([128, tc], F32, tag="po")
                nc.scalar.activation(pot[:], pso[:], AF.Identity,
                                     bias=bo_sb[:, eb:eb + 1])
                nc.sync.dma_start(part[eb * 128:(eb + 1) * 128, t0:t0 + tc],
                                  pot[:])
    nc.compile()
    return nc


def _build_stage2(t_len=T):
    nc = bacc.Bacc("TRN2", target_bir_lowering=False, debug=False, num_devices=8)
    p0 = nc.dram_tensor("p0", [DL, t_len], F32, kind="ExternalInput")
    p1 = nc.dram_tensor("p1", [DL, t_len], F32, kind="ExternalInput")
    s = nc.dram_tensor("s", [DL, t_len], F32, kind="ExternalOutput")
    tcw = 512
    with tile.TileContext(nc) as tc_ctx, ExitStack() as ctx:
        pool = ctx.enter_context(tc_ctx.tile_pool(name="p", bufs=6))
        for kb in range(DL // 128):
            for i in range(t_len // tcw):
                t0 = i * tcw
                a_t = pool.tile([128, tcw], F32, tag="a")
                nc.sync.dma_start(a_t[:], p0[kb * 128:(kb + 1) * 128, t0:t0 + tcw])
                b_t = pool.tile([128, tcw], F32, tag="b")
                nc.sync.dma_start(b_t[:], p1[kb * 128:(kb + 1) * 128, t0:t0 + tcw])
                o_t = pool.tile([128, tcw], F32, tag="o")
                nc.vector.tensor_add(o_t[:], a_t[:], b_t[:])
                nc.sync.dma_start(s[kb * 128:(kb + 1) * 128, t0:t0 + tcw], o_t[:])
    nc.compile()
    return nc


def _stage1_inputs(x, A_log, Dp, W_xproj, b_xproj, W_out, b_out):
    bf = ml_dtypes.bfloat16
    in_maps = []
    for c in range(8):
        b, j = c % 4, c // 4
        lo, hi = j * DL, (j + 1) * DL
        order = np.concatenate(
            [np.arange(lo, hi), np.arange(0, lo), np.arange(hi, D)])
        cols = np.concatenate([np.arange(lo, hi), np.arange(D, D + 2 * N)])
        xt_full = np.ascontiguousarray(x[b].T[order]).astype(bf)
        wxc = np.ascontiguousarray(W_xproj[order][:, cols]).astype(bf)
        bx_pad = np.zeros(5 * 128, np.float32)
        bx_pad[:PC] = b_xproj[cols]
        bx_arr = np.ascontiguousarray(bx_pad.reshape(5, 128).T)
        alog_l = np.ascontiguousarray(
            A_log[lo:hi].reshape(NJB, 128, N).transpose(1, 0, 2).reshape(128, NJB * N))
        dp_l = np.ascontiguousarray(Dp[lo:hi].reshape(NJB, 128).T)
        wo_l = np.ascontiguousarray(W_out[lo:hi]).astype(bf)
        bo_src = b_out if j == 0 else np.zeros_like(b_out)
        bo_l = np.ascontiguousarray(bo_src.reshape(NEB, 128).T.astype(np.float32))
        in_maps.append({
            "xt": xt_full, "wx": wxc, "bx": bx_arr, "alog": alog_l,
            "dp": dp_l, "wo": wo_l, "bo": bo_l,
        })
    return in_maps


def kernel(x, A_log, Dp, W_xproj, b_xproj, W_out, b_out, _trace=False):
    x = np.asarray(x, np.float32)
    A_log = np.asarray(A_log, np.float32)
    Dp = np.asarray(Dp, np.float32)
    W_xproj = np.asarray(W_xproj, np.float32)
    b_xproj = np.asarray(b_xproj, np.float32)
    W_out = np.asarray(W_out, np.float32)
    b_out = np.asarray(b_out, np.float32)

    if "s1" not in _cache:
        _cache["s1"] = _build_stage1()
    if "s2" not in _cache:
        _cache["s2"] = _build_stage2()

    in1 = _stage1_inputs(x, A_log, Dp, W_xproj, b_xproj, W_out, b_out)
    kw = dict(trace=True, trace_cores=list(range(8))) if _trace else {}
    res1 = run_bass_kernel_spmd(_cache["s1"], in1, core_ids=list(range(8)), **kw)
    parts = [res1.results[c]["part"] for c in range(8)]

    in2 = []
    for c in range(8):
        b, eh = c % 4, c // 4
        in2.append({
            "p0": np.ascontiguousarray(parts[b][eh * DL:(eh + 1) * DL]),
            "p1": np.ascontiguousarray(parts[4 + b][eh * DL:(eh + 1) * DL]),
        })
    res2 = run_bass_kernel_spmd(_cache["s2"], in2, core_ids=list(range(8)), **kw)

    outs = []
    for b in range(4):
        s0 = res2.results[b]["s"]
        s1 = res2.results[4 + b]["s"]
        outs.append(np.concatenate([s0, s1], axis=0).T)
    out = np.stack(outs).astype(np.float32)
    if _trace:
        return out, (res1, res2)
    return out
